# revision 1
# baseline (speedup 1.0000x reference)
"""Block-sparse linear kernel for Trainium2 (8 NeuronCores, SPMD).

Computes out = x @ W.T + bias where W is a 4096x4096 block-sparse matrix
given as 8192 active 32x32 blocks (50% density).

Strategy:
  - Data-parallel over tokens: 8192 tokens -> 1024 per core; weights replicated.
  - On device, compute out.T = W @ x.T with dense TensorE matmuls
    (the 32x32 random sparsity cannot beat the dense array roofline on TRN2:
    sub-array packed matmuls are weight-load-port bound at ~34ns/block,
    2x worse than the dense stream), accumulate in fp32 PSUM, fused bias
    add on psum evacuation, DMA out.
  - Matmul dtype selectable: fp16 (default) or float32r (~tf32). Both run
    at 1 moving-column/cycle; fp16 halves DMA/SBUF footprint.
  - Host densifies/pre-transposes weights into SBUF-image layout and
    transposes x/out (cheap numpy work, off the device critical path).
"""

import os
import numpy as np

import concourse.bacc as bacc
import concourse.mybir as mybir
import concourse.tile as tile
from concourse.bass_utils import run_bass_kernel_spmd

SLIM_TAIL = os.environ.get("KERNEL_SLIM_TAIL", "0") == "1"
if SLIM_TAIL:
    from concourse.vector_clock import ScopedClock as _ScopedClock

    def _slim_drain_and_barrier(self, tick_clock, wait_clock):
        # Same as TileContext._drain_and_barrier but without the trailing
        # all-engine barrier: each engine's sem clears are ordered before
        # NEFF completion by its own program order, so re-execution still
        # sees cleared semaphores. Saves ~3.5us of kernel tail.
        drain_inst = self.nc.sync.drain()
        wait_clock.add_sem_waits(
            drain_inst.ins, _ScopedClock({None: tick_clock.global_clock})
        )
        self.nc.all_engine_barrier()
        popped = self.nc._tile_sem_poison_stack.pop()
        assert popped is self._sem_poison
        self.nc.clear_and_free_semaphores(list(self.sems.allocated().values()))

    tile.TileContext._drain_and_barrier = _slim_drain_and_barrier

TOKENS = 8192
IN = 4096
OUT = 4096
BS = 32
NBR = OUT // BS   # 128 block rows
NBC = IN // BS    # 128 block cols
NCORES = 8
TPC = TOKENS // NCORES   # 1024 tokens per core

MCH = 128   # output chunk (psum partitions)
KCH = 128   # contraction chunk (sbuf partitions)
NCH = 512   # token chunk (psum free dim, one bank of fp32)
NM = OUT // MCH    # 32
NK = IN // KCH     # 32
NN = TPC // NCH    # 2

DTYPE = os.environ.get("KERNEL_DTYPE", "f16")   # f16 | f32r
WBUFS = int(os.environ.get("KERNEL_WBUFS", "5"))
PSUM_BUFS = int(os.environ.get("KERNEL_PSUM_BUFS", "7"))
WARM_MMS = int(os.environ.get("KERNEL_WARM_MMS", "80"))
INTER = int(os.environ.get("KERNEL_INTER", "0"))
DEFER = os.environ.get("KERNEL_DEFER", "0") == "1"

_CACHE: dict = {}


def _mdt():
    return mybir.dt.float16 if DTYPE == "f16" else mybir.dt.float32r


def _npdt():
    return np.float16 if DTYPE == "f16" else np.float32


def _build_dense():
    """Dense matmul module: out.T[m] = sum_k W.T[k,m].T @ x.T[k] + bias."""
    mdt = _mdt()
    nc = bacc.Bacc("TRN2", target_bir_lowering=False, debug=False)

    wt = nc.dram_tensor("wt", [NM, KCH, NK * MCH], mdt, kind="ExternalInput")
    xt = nc.dram_tensor("xt", [NK, KCH, TPC], mdt, kind="ExternalInput")
    bias_img = nc.dram_tensor("bias_img", [MCH, NM], mybir.dt.float32,
                              kind="ExternalInput")
    outT = nc.dram_tensor("outT", [NM, MCH, TPC], mybir.dt.float32,
                          kind="ExternalOutput")

    with tile.TileContext(nc) as tc:
        with (
            tc.tile_pool(name="xres", bufs=NK * NN) as xres,
            tc.tile_pool(name="wbuf", bufs=WBUFS) as wbuf,
            tc.tile_pool(name="obuf", bufs=4) as obuf,
            tc.tile_pool(name="misc", bufs=1) as misc,
            tc.tile_pool(name="ps", bufs=PSUM_BUFS, space="PSUM") as ps,
        ):
            bias_t = misc.tile([MCH, NM], mybir.dt.float32, tag="bias")
            nc.scalar.dma_start(bias_t[:], bias_img.ap())

            # PE warm-up: the HAM clock gate keeps the array at 1.2 GHz until
            # ~3.4us of sustained activity. Run throwaway matmuls on a local
            # zeroed tile during the initial DMA wait so real matmuls start
            # at 2.4 GHz and the PE never sits idle past a MID window.
            if WARM_MMS:
                # The source tile must be written before the PE reads it:
                # matmul on never-written SBUF wedges the device (parity).
                wz = misc.tile([KCH, MCH], mdt, tag="wz")
                nc.gpsimd.memset(wz[:], 0.0)
                pwarm = ps.tile([MCH, 64], mybir.dt.float32, tag="pw",
                                name="pwarm", bufs=1)
                for _ in range(WARM_MMS):
                    nc.tensor.matmul(pwarm[:], wz[:], wz[:, :64],
                                     start=True, stop=True)

            # x halves [k, n] on the ACT HWDGE ring, ordered n-major so the
            # n=0 sweep's data (4MB) lands first; W/out use the SP ring.
            xtiles = {}
            for n in range(NN):
                for k in range(NK):
                    t = xres.tile([KCH, NCH], mdt, tag="x", name=f"x{k}_{n}")
                    if n == 0 or not DEFER:
                        nc.scalar.dma_start(t[:], xt.ap()[k][:, n * NCH:(n + 1) * NCH])
                    xtiles[(k, n)] = t

            # Head phase: while x is still streaming in, run the first INTER
            # m-chunks of n=0 k-outer (INTER matmuls per arriving x tile) so
            # the PE keeps pace with DMA arrival instead of stalling.
            if INTER:
                ws, ps_head = [], []
                for m in range(INTER):
                    w = wbuf.tile([KCH, NK * MCH], mdt, tag="w", name=f"wh{m}")
                    ws.append(w)
                    p = ps.tile([MCH, NCH], mybir.dt.float32, tag="p",
                                name=f"ph{m}")
                    ps_head.append(p)
                for c in range(4):
                    cs = c * (NK // 4) * MCH
                    ce = (c + 1) * (NK // 4) * MCH
                    for m in range(INTER):
                        nc.sync.dma_start(ws[m][:, cs:ce], wt.ap()[m][:, cs:ce])
                for k in range(NK):
                    for m in range(INTER):
                        nc.tensor.matmul(
                            ps_head[m][:],
                            ws[m][:, k * MCH:(k + 1) * MCH],
                            xtiles[(k, 0)][:],
                            start=(k == 0),
                            stop=(k == NK - 1),
                        )
                for m in range(INTER):
                    o = obuf.tile([MCH, NCH], mybir.dt.float32, tag="o",
                                  name=f"oh{m}")
                    nc.vector.tensor_scalar_add(o[:], ps_head[m][:],
                                                bias_t[:, m:m + 1])
                    nc.sync.dma_start(outT.ap()[m][:, 0:NCH], o[:])

            # n-outer: W is streamed once per n-chunk (2x total) so the
            # first psum group only waits for the first x half-tiles.
            for n in range(NN):
                for m in range(INTER if n == 0 else 0, NM):
                    w = wbuf.tile([KCH, NK * MCH], mdt, tag="w", name=f"w{n}_{m}")
                    for c in range(4):
                        cs = c * (NK // 4) * MCH
                        ce = (c + 1) * (NK // 4) * MCH
                        nc.sync.dma_start(w[:, cs:ce], wt.ap()[m][:, cs:ce])
                    if DEFER and n == 0 and m == 8:
                        for kk in range(NK):
                            nc.sync.dma_start(xtiles[(kk, 1)][:],
                                              xt.ap()[kk][:, NCH:2 * NCH])
                    p = ps.tile([MCH, NCH], mybir.dt.float32, tag="p",
                                name=f"p{n}_{m}")
                    for k in range(NK):
                        nc.tensor.matmul(
                            p[:],
                            w[:, k * MCH:(k + 1) * MCH],
                            xtiles[(k, n)][:],
                            start=(k == 0),
                            stop=(k == NK - 1),
                        )
                    o = obuf.tile([MCH, NCH], mybir.dt.float32, tag="o",
                                  name=f"o{n}_{m}")
                    nc.vector.tensor_scalar_add(o[:], p[:], bias_t[:, m:m + 1])
                    nc.sync.dma_start(outT.ap()[m][:, n * NCH:(n + 1) * NCH], o[:])

    nc.compile()
    return nc


def _get_nc():
    if "nc" not in _CACHE:
        _CACHE["nc"] = _build_dense()
    return _CACHE["nc"]


def _densify(weight_data, block_rows, block_cols):
    """Scatter 32x32 blocks into dense W (OUT, IN)."""
    w4 = np.zeros((NBR, NBC, BS, BS), dtype=np.float32)
    w4[block_rows, block_cols] = weight_data
    return w4.transpose(0, 2, 1, 3).reshape(OUT, IN)


def _make_in_maps(x, weight_data, bias, block_rows, block_cols):
    ndt = _npdt()
    W = _densify(np.asarray(weight_data, dtype=np.float32),
                 np.asarray(block_rows), np.asarray(block_cols))
    # wt[m][i2, k*128+o2] = W[m*128+o2, k*128+i2]
    wt = np.ascontiguousarray(
        W.reshape(NM, MCH, NK, KCH).transpose(0, 3, 2, 1).astype(ndt)
    ).reshape(NM, KCH, NK * MCH)
    # xt[core][k][i2, t] = x[core*TPC+t, k*128+i2]
    xt_all = np.ascontiguousarray(
        np.asarray(x, dtype=np.float32)
        .reshape(NCORES, TPC, NK, KCH).transpose(0, 2, 3, 1).astype(ndt)
    )
    bias_img = np.ascontiguousarray(
        np.asarray(bias, dtype=np.float32).reshape(NM, MCH).T
    )
    return [
        {"wt": wt, "xt": xt_all[c], "bias_img": bias_img}
        for c in range(NCORES)
    ]


def _assemble(results):
    out = np.empty((TOKENS, OUT), dtype=np.float32)
    for c, r in enumerate(results):
        out[c * TPC:(c + 1) * TPC] = r["outT"].reshape(OUT, TPC).T
    return out


def kernel(x, weight_data, bias, block_rows, block_cols):
    nc = _get_nc()
    in_maps = _make_in_maps(x, weight_data, bias, block_rows, block_cols)
    res = run_bass_kernel_spmd(nc, in_maps, core_ids=list(range(NCORES)))
    return _assemble(res.results)



# revision 2
# speedup vs baseline: 1.1493x; 1.1493x over previous
"""Block-sparse linear kernel for Trainium2 (8 NeuronCores, SPMD).

Computes out = x @ W.T + bias where W is a 4096x4096 block-sparse matrix
given as 8192 active 32x32 blocks (50% density).

Strategy:
  - Data-parallel over tokens: 8192 tokens -> 1024 per core; weights replicated.
  - On device, compute out.T = W @ x.T with dense TensorE matmuls
    (the 32x32 random sparsity cannot beat the dense array roofline on TRN2:
    sub-array packed matmuls are weight-load-port bound at ~34ns/block,
    2x worse than the dense stream), accumulate in fp32 PSUM, fused bias
    add on psum evacuation, DMA out.
  - Mixed precision: P k-tile-pairs (2P of 32 k-tiles) run as fp8e4
    DoubleRow matmuls (K=256 per MM at the same 216ns/MM as a K=128 fp16
    MM -> true 2x); the remaining k-tiles run fp16. P=4 (alpha=0.25)
    measures rel_norm 1.6e-2 on the seed-0 data (gate 2e-2).
  - Everything scaled by S=64 on host (W*64, bias*64) so fp8 W avoids
    e4m3 subnormals; host divides the output by 64.
  - Host densifies/pre-transposes weights into SBUF-image layout and
    transposes x/out (cheap numpy work, off the device critical path).
"""

import os
import numpy as np
import ml_dtypes

import concourse.bacc as bacc
import concourse.mybir as mybir
import concourse.tile as tile
from concourse.bass_utils import run_bass_kernel_spmd

SLIM_TAIL = os.environ.get("KERNEL_SLIM_TAIL", "1") == "1"
if SLIM_TAIL:
    from concourse.vector_clock import ScopedClock as _ScopedClock

    def _slim_drain_and_barrier(self, tick_clock, wait_clock):
        # Same as TileContext._drain_and_barrier but without the trailing
        # all-engine barrier: each engine's sem clears are ordered before
        # NEFF completion by its own program order, so re-execution still
        # sees cleared semaphores. Saves ~3.5us of kernel tail.
        drain_inst = self.nc.sync.drain()
        wait_clock.add_sem_waits(
            drain_inst.ins, _ScopedClock({None: tick_clock.global_clock})
        )
        self.nc.all_engine_barrier()
        popped = self.nc._tile_sem_poison_stack.pop()
        assert popped is self._sem_poison
        self.nc.clear_and_free_semaphores(list(self.sems.allocated().values()))

    tile.TileContext._drain_and_barrier = _slim_drain_and_barrier

TOKENS = 8192
IN = 4096
OUT = 4096
BS = 32
NBR = OUT // BS   # 128 block rows
NBC = IN // BS    # 128 block cols
NCORES = 8
TPC = TOKENS // NCORES   # 1024 tokens per core

MCH = 128   # output chunk (psum partitions)
KCH = 128   # contraction chunk (sbuf partitions)
NCH = 512   # token chunk (psum free dim, one bank of fp32)
NM = OUT // MCH    # 32
NK = IN // KCH     # 32 total k-tiles
NN = TPC // NCH    # 2

P8 = int(os.environ.get("KERNEL_P8", "4"))     # fp8 DoubleRow k-tile PAIRS
NK16 = NK - 2 * P8                             # fp16 k-tiles
SCALE = 64.0

WBUFS = int(os.environ.get("KERNEL_WBUFS", "5"))
PSUM_BUFS = int(os.environ.get("KERNEL_PSUM_BUFS", "7"))
WARM_MMS = int(os.environ.get("KERNEL_WARM_MMS", "80"))
DEFER = os.environ.get("KERNEL_DEFER", "0") == "1"

_CACHE: dict = {}

F16 = mybir.dt.float16
FP8 = mybir.dt.float8e4
F32 = mybir.dt.float32


def _build():
    """Mixed fp8-DoubleRow / fp16 matmul module:
    out.T[m] = sum_kp W8[kp,m].T @ x8.T[kp] + sum_k W16[k,m].T @ x16.T[k],
    all scaled by 64; bias64 added on psum evacuation."""
    nc = bacc.Bacc("TRN2", target_bir_lowering=False, debug=False)

    wt16 = nc.dram_tensor("wt16", [NM, KCH, NK16 * MCH], F16,
                          kind="ExternalInput")
    xt16 = nc.dram_tensor("xt16", [NK16, KCH, TPC], F16, kind="ExternalInput")
    if P8:
        wt8 = nc.dram_tensor("wt8", [NM, KCH, P8, 2, MCH], FP8,
                             kind="ExternalInput")
        xt8 = nc.dram_tensor("xt8", [P8, KCH, 2, TPC], FP8,
                             kind="ExternalInput")
    bias_img = nc.dram_tensor("bias_img", [MCH, NM], F32, kind="ExternalInput")
    outT = nc.dram_tensor("outT", [NM, MCH, TPC], F32, kind="ExternalOutput")

    DR = mybir.MatmulPerfMode.DoubleRow

    with tile.TileContext(nc) as tc:
        with (
            tc.tile_pool(name="xres", bufs=NK16 * NN) as xres,
            tc.tile_pool(name="xres8", bufs=max(P8, 1) * NN) as xres8,
            tc.tile_pool(name="wbuf", bufs=WBUFS) as wbuf,
            tc.tile_pool(name="wbuf8", bufs=WBUFS) as wbuf8,
            tc.tile_pool(name="obuf", bufs=4) as obuf,
            tc.tile_pool(name="misc", bufs=1) as misc,
            tc.tile_pool(name="ps", bufs=PSUM_BUFS, space="PSUM") as ps,
        ):
            bias_t = misc.tile([MCH, NM], F32, tag="bias")
            nc.scalar.dma_start(bias_t[:], bias_img.ap())

            # PE warm-up: the HAM clock gate keeps the array at 1.2 GHz until
            # ~3.4us of sustained activity. Run throwaway matmuls on a local
            # zeroed tile during the initial DMA wait so real matmuls start
            # at 2.4 GHz and the PE never sits idle past a MID window.
            if WARM_MMS:
                # The source tile must be written before the PE reads it:
                # matmul on never-written SBUF wedges the device (parity).
                wz = misc.tile([KCH, MCH], F16, tag="wz")
                nc.gpsimd.memset(wz[:], 0.0)
                pwarm = ps.tile([MCH, 64], F32, tag="pw",
                                name="pwarm", bufs=1)
                for _ in range(WARM_MMS):
                    nc.tensor.matmul(pwarm[:], wz[:], wz[:, :64],
                                     start=True, stop=True)

            # x on the ACT HWDGE ring, ordered n-major so the n=0 sweep's
            # data lands first; W/out use the SP ring. fp8 tiles first (they
            # gate the DR matmuls that open each psum group).
            xtiles16 = {}
            xtiles8 = {}
            for n in range(NN):
                for kp in range(P8):
                    t = xres8.tile([KCH, 2, NCH], FP8, tag="x8",
                                   name=f"x8_{kp}_{n}")
                    if n == 0 or not DEFER:
                        nc.scalar.dma_start(
                            t[:], xt8.ap()[kp][:, :, n * NCH:(n + 1) * NCH])
                    xtiles8[(kp, n)] = t
                for k in range(NK16):
                    t = xres.tile([KCH, NCH], F16, tag="x", name=f"x{k}_{n}")
                    if n == 0 or not DEFER:
                        nc.scalar.dma_start(
                            t[:], xt16.ap()[k][:, n * NCH:(n + 1) * NCH])
                    xtiles16[(k, n)] = t

            # n-outer: W is streamed once per n-chunk (2x total) so the
            # first psum group only waits for the first x half-tiles.
            for n in range(NN):
                for m in range(NM):
                    if P8:
                        w8 = wbuf8.tile([KCH, P8, 2, MCH], FP8, tag="w8",
                                        name=f"w8_{n}_{m}")
                        nc.sync.dma_start(w8[:], wt8.ap()[m])
                    w = wbuf.tile([KCH, NK16 * MCH], F16, tag="w",
                                  name=f"w{n}_{m}")
                    for c in range(4):
                        cs = c * (NK16 * MCH // 4)
                        ce = (c + 1) * (NK16 * MCH // 4)
                        nc.sync.dma_start(w[:, cs:ce], wt16.ap()[m][:, cs:ce])
                    if DEFER and n == 0 and m == 8:
                        for kk in range(P8):
                            nc.sync.dma_start(
                                xtiles8[(kk, 1)][:],
                                xt8.ap()[kk][:, :, NCH:2 * NCH])
                        for kk in range(NK16):
                            nc.sync.dma_start(xtiles16[(kk, 1)][:],
                                              xt16.ap()[kk][:, NCH:2 * NCH])
                    p = ps.tile([MCH, NCH], F32, tag="p", name=f"p{n}_{m}")
                    for kp in range(P8):
                        nc.tensor.matmul(
                            p[:],
                            w8[:, kp],
                            xtiles8[(kp, n)][:],
                            start=(kp == 0),
                            stop=False,
                            perf_mode=DR,
                        )
                    for k in range(NK16):
                        nc.tensor.matmul(
                            p[:],
                            w[:, k * MCH:(k + 1) * MCH],
                            xtiles16[(k, n)][:],
                            start=(k == 0 and P8 == 0),
                            stop=(k == NK16 - 1),
                        )
                    o = obuf.tile([MCH, NCH], F32, tag="o", name=f"o{n}_{m}")
                    nc.vector.tensor_scalar_add(o[:], p[:], bias_t[:, m:m + 1])
                    nc.sync.dma_start(outT.ap()[m][:, n * NCH:(n + 1) * NCH], o[:])

    nc.compile()
    return nc


def _get_nc():
    if "nc" not in _CACHE:
        _CACHE["nc"] = _build()
    return _CACHE["nc"]


def _densify(weight_data, block_rows, block_cols):
    """Scatter 32x32 blocks into dense W (OUT, IN)."""
    w4 = np.zeros((NBR, NBC, BS, BS), dtype=np.float32)
    w4[block_rows, block_cols] = weight_data
    return w4.transpose(0, 2, 1, 3).reshape(OUT, IN)


def _make_in_maps(x, weight_data, bias, block_rows, block_cols):
    W = _densify(np.asarray(weight_data, dtype=np.float32),
                 np.asarray(block_rows), np.asarray(block_cols)) * SCALE
    x = np.asarray(x, dtype=np.float32)
    KF8 = 2 * P8 * KCH   # fp8 k-range (features 0..KF8)

    # fp8 part: wt8[m][i][kp][t][o] = W[m*128+o, (2kp+t)*128+i]
    W8 = W[:, :KF8].astype(ml_dtypes.float8_e4m3)
    wt8 = np.ascontiguousarray(
        W8.reshape(NM, MCH, P8, 2, KCH).transpose(0, 4, 2, 3, 1))
    # xt8[core][kp][i][t][n] = x[core*TPC+n, (2kp+t)*128+i]
    x8 = x[:, :KF8].astype(ml_dtypes.float8_e4m3)
    xt8_all = np.ascontiguousarray(
        x8.reshape(NCORES, TPC, P8, 2, KCH).transpose(0, 2, 4, 3, 1))

    # fp16 part: wt16[m][i2, k*128+o2] = W[m*128+o2, KF8 + k*128+i2]
    W16 = W[:, KF8:].astype(np.float16)
    wt16 = np.ascontiguousarray(
        W16.reshape(NM, MCH, NK16, KCH).transpose(0, 3, 2, 1)
    ).reshape(NM, KCH, NK16 * MCH)
    # xt16[core][k][i2, t] = x[core*TPC+t, KF8 + k*128+i2]
    xt16_all = np.ascontiguousarray(
        x[:, KF8:]
        .reshape(NCORES, TPC, NK16, KCH).transpose(0, 2, 3, 1)
        .astype(np.float16)
    )
    bias_img = np.ascontiguousarray(
        (np.asarray(bias, dtype=np.float32) * SCALE).reshape(NM, MCH).T
    )
    maps = []
    for c in range(NCORES):
        m = {"wt16": wt16, "xt16": xt16_all[c], "bias_img": bias_img}
        if P8:
            m["wt8"] = wt8
            m["xt8"] = xt8_all[c]
        maps.append(m)
    return maps


def _assemble(results):
    out = np.empty((TOKENS, OUT), dtype=np.float32)
    inv = np.float32(1.0 / SCALE)
    for c, r in enumerate(results):
        out[c * TPC:(c + 1) * TPC] = r["outT"].reshape(OUT, TPC).T * inv
    return out


def kernel(x, weight_data, bias, block_rows, block_cols):
    nc = _get_nc()
    in_maps = _make_in_maps(x, weight_data, bias, block_rows, block_cols)
    res = run_bass_kernel_spmd(nc, in_maps, core_ids=list(range(NCORES)))
    return _assemble(res.results)
